# revision 9
# baseline (speedup 1.0000x reference)
"""BoxFilter (9x9 unnormalized box sum, zero-padded borders) on 8 trn2 cores.

Full input: image [8, 32, 512, 512] f32. Sharded batch-wise: core b handles
image[b] = [32, 512, 512]. Per (channel) slice X [512, 512]:

    out = Band @ X @ Band,   Band[p, q] = 1 iff |p - q| <= RADIUS

computed as two PE passes that each transpose, so the result lands back in
[h, w] orientation:

    pass A:  YT[w, i] = sum_j X[j, w]   * Band[j, i]   (lhsT = X chunk,  rhs = Band)
    pass B:  O[i, w']  = sum_w YT[w, i] * Band[w, w']  (lhsT = YT chunk, rhs = Band)

Band is block-Toeplitz; per 128-row chunk t its column support is
[128t-4, 128t+132). The first accumulating matmul covers the full 512-wide
PSUM bank (clearing has_written everywhere); chunks 1-3 ride 256-wide
windows that contain their support (fp32r needs moving dim >= 256 for
1 cycle/row).

Matmuls run in fp32r (TF32-like: 11-bit mantissa). Inputs are pre-rounded to
the fp32r grid on the host so the device interpretation is exact; the PSUM
accumulator stays fp32. Max rel err vs the fp32 reference ~1.5e-4.
"""

import numpy as np

import concourse.bass as bass
import concourse.mybir as mybir
import concourse.tile as tile
from concourse import bacc, bass_utils

RADIUS = 4
H = W = 512
P = 128  # partitions / chunk size
NCHUNK = H // P  # 4
N_CORES = 8
NCH = 32  # channels per core (batch dim sharded across cores)

# moving-window offsets per chunk; chunk 0 uses the full 512 (start=True pass
# must cover the whole PSUM bank so later windowed accumulates see uniform
# has_written state)
WIN_OFF = [0, 64, 192, 256]
WIN_N = [512, 256, 256, 256]
# column offset of chunk t's slab inside the packed band constant
BAND_COL = [0, 512, 768, 1024]
BAND_TOT = 1280


def round_to_fp32r(a: np.ndarray) -> np.ndarray:
    """Round fp32 to the fp32r grid (8-bit exp, 11-bit mantissa: RNE, low 12
    bits zeroed) so the on-device fp32r interpretation is exact."""
    u = np.ascontiguousarray(a).view(np.uint32)
    lsb = (u >> np.uint32(12)) & np.uint32(1)
    r = (u + np.uint32(0x7FF) + lsb) & np.uint32(0xFFFFF000)
    return r.view(np.float32)


def band_constant() -> np.ndarray:
    """Packed [128, 1280] f32: chunk 0 full width, chunks 1-3 as 256 windows."""
    q = np.arange(H)
    out = np.zeros((P, BAND_TOT), dtype=np.float32)
    for t in range(NCHUNK):
        rows = np.arange(P * t, P * t + P)
        blk = (np.abs(rows[:, None] - q[None, :]) <= RADIUS).astype(np.float32)
        out[:, BAND_COL[t] : BAND_COL[t] + WIN_N[t]] = blk[
            :, WIN_OFF[t] : WIN_OFF[t] + WIN_N[t]
        ]
    return out


def _emit_pass(nc, pools, band_r, x_ap, y_ap, nch, in_is_f32r, scale=None):
    """Emit the full boxfilter for one [nch, H, W] tensor pair."""
    f32 = mybir.dt.float32
    f32r = mybir.dt.float32r
    const_pool, x_pool, yt_pool, o_pool, psA, psB = pools
    for c in range(nch):
        xt = []
        for t in range(NCHUNK):
            xtile = x_pool.tile([P, W], f32r, tag="x")
            src = x_ap[c, P * t : P * t + P, :]
            if not in_is_f32r:
                src = src.bitcast(f32r)
            nc.sync.dma_start(xtile[:], src)
            xt.append(xtile)

        yts = []
        for b in range(NCHUNK):  # w-block
            yt_ps = psA.tile([P, W], f32)
            for t in range(NCHUNK):  # j-chunk (contract over h)
                nc.tensor.matmul(
                    yt_ps[:, WIN_OFF[t] : WIN_OFF[t] + WIN_N[t]],
                    lhsT=xt[t][:, P * b : P * b + P],
                    rhs=band_r[t],
                    start=(t == 0),
                    stop=(t == NCHUNK - 1),
                )
            yt_sb = yt_pool.tile([P, W], f32r, tag="yt")
            nc.vector.tensor_copy(yt_sb[:], yt_ps[:])
            yts.append(yt_sb)

        for d in range(NCHUNK):  # h-block of the final output
            o_ps = psB.tile([P, W], f32)
            for s in range(NCHUNK):  # w-chunk (contract over w)
                nc.tensor.matmul(
                    o_ps[:, WIN_OFF[s] : WIN_OFF[s] + WIN_N[s]],
                    lhsT=yts[s][:, P * d : P * d + P],
                    rhs=band_r[s],
                    start=(s == 0),
                    stop=(s == NCHUNK - 1),
                )
            o_sb = o_pool.tile([P, W], f32, tag="o")
            if scale is None:
                nc.scalar.copy(o_sb[:], o_ps[:])
            else:
                nc.scalar.mul(o_sb[:], o_ps[:], scale)
            nc.sync.dma_start(y_ap[c, P * d : P * d + P, :], o_sb[:])


def build_nc(nch: int = NCH, chain: int = 1):
    """chain > 1 repeats the filter through internal DRAM scratch (for
    benchmarking: the K-difference isolates pure device time)."""
    f32 = mybir.dt.float32
    f32r = mybir.dt.float32r
    nc = bacc.Bacc("TRN2", target_bir_lowering=False, debug=False)
    x = nc.dram_tensor("x", [nch, H, W], f32r, kind="ExternalInput").ap()
    band_d = nc.dram_tensor("band", [P, BAND_TOT], f32r, kind="ExternalInput").ap()
    y = nc.dram_tensor("y", [nch, H, W], f32, kind="ExternalOutput").ap()

    with tile.TileContext(nc) as tc:
        with (
            tc.tile_pool(name="const", bufs=1) as const_pool,
            tc.tile_pool(name="xin", bufs=8) as x_pool,
            tc.tile_pool(name="yt", bufs=8) as yt_pool,
            tc.tile_pool(name="osb", bufs=4) as o_pool,
            tc.tile_pool(name="psA", bufs=3, space="PSUM") as psA,
            tc.tile_pool(name="psB", bufs=3, space="PSUM") as psB,
            tc.tile_pool(name="dram", bufs=2, space="DRAM") as dram_pool,
        ):
            band_sb = const_pool.tile([P, BAND_TOT], f32r)
            nc.sync.dma_start(band_sb[:], band_d[:])
            band_r = [
                band_sb[:, BAND_COL[t] : BAND_COL[t] + WIN_N[t]]
                for t in range(NCHUNK)
            ]
            pools = (const_pool, x_pool, yt_pool, o_pool, psA, psB)

            scale = None if chain == 1 else 1.0 / 81.0
            cur = x
            cur_f32r = True
            for it in range(chain):
                last = it == chain - 1
                dst = (
                    y
                    if last
                    else dram_pool.tile([nch, H, W], f32, tag="scratch")
                )
                _emit_pass(nc, pools, band_r, cur, dst, nch, cur_f32r, scale)
                cur = dst
                cur_f32r = False

    nc.compile()
    return nc


def kernel(image) -> np.ndarray:
    image = np.ascontiguousarray(np.asarray(image, dtype=np.float32))
    assert image.shape == (N_CORES, NCH, H, W), image.shape
    image = round_to_fp32r(image)
    nc = build_nc(NCH)
    band = band_constant()
    in_maps = [{"x": image[b], "band": band} for b in range(N_CORES)]
    res = bass_utils.run_bass_kernel_spmd(nc, in_maps, core_ids=list(range(N_CORES)))
    return np.stack([r["y"] for r in res.results], axis=0)


if __name__ == "__main__":
    img = np.random.rand(N_CORES, NCH, H, W).astype(np.float32)
    out = kernel(img)
    print(out.shape, out.dtype)
